# revision 11
# baseline (speedup 1.0000x reference)
"""L0-gated SINDy reward kernel for TRN2 (8 NeuronCores, data-parallel).

out[b] = sum_j c_j * m_j(x_b) with x = concat(obs, act) [B, 4],
m_j = 35 monomials of degree <= 3 (sklearn PolynomialFeatures order),
c_j = clip(sigmoid(qz_loga)*1.2 - 0.1, 0, 1) * weights[:, 0].

Host folds gate*weight into 35 scalars (compile-time immediates) and packs
x per core as [128, 4, 1024] bf16. On-chip (raw bass, explicit sems):
Horner in d(=act) with fused scalar_tensor_tensor MACs on DVE, squares on
ACT, double-buffered DMA on sync.
"""

import numpy as np
import ml_dtypes

B = 1048576
NCORES = 8
R = B // NCORES          # rows per core
P = 128
FTOT = R // P            # 1024 free elems per partition
FS = 512                 # free size per chunk
NT = FTOT // FS

GAMMA, ZETA = -0.1, 1.1
COMPUTE = "bf16"

_CACHED = {}


def _build_nc(c):
    import concourse.bass as bass
    import concourse.mybir as mybir
    from contextlib import ExitStack

    f32 = mybir.dt.float32
    bf16 = mybir.dt.bfloat16 if COMPUTE == "bf16" else mybir.dt.float32
    MUL = mybir.AluOpType.mult
    ADD = mybir.AluOpType.add

    c = [float(v) for v in c]

    nc = bass.Bass()
    X = nc.dram_tensor("X", [P, 4, FTOT], bf16, kind="ExternalInput")
    out_d = nc.dram_tensor("out", [P, FTOT], f32, kind="ExternalOutput")

    with ExitStack() as ctx:
        def sb(nm, shape, dt):
            return ctx.enter_context(nc.sbuf_tensor(nm, shape, dt))

        XT = [sb(f"XT{i}", [P, 4, FS], bf16) for i in range(NT)]
        AAs = [sb(f"AAs{i}", [P, FS], bf16) for i in range(NT)]
        BBs = [sb(f"BBs{i}", [P, FS], bf16) for i in range(NT)]
        CCs = [sb(f"CCs{i}", [P, FS], bf16) for i in range(NT)]
        AB = sb("AB", [P, FS], bf16)
        AC = sb("AC", [P, FS], bf16)
        BC = sb("BC", [P, FS], bf16)
        t = sb("t", [P, FS], bf16)
        Ra = sb("Ra", [P, FS], bf16)
        s = sb("s", [P, FS], bf16)
        Rb = sb("Rb", [P, FS], bf16)
        u = sb("u", [P, FS], bf16)
        ot = [sb(f"ot{i}", [P, FS], f32) for i in range(NT)]

        dsem = ctx.enter_context(nc.semaphore())
        asem = ctx.enter_context(nc.semaphore())
        vsem = ctx.enter_context(nc.semaphore())
        osem = ctx.enter_context(nc.semaphore())
        gsem = ctx.enter_context(nc.semaphore())
        block = ctx.enter_context(nc.Block())

        @block.gpsimd
        def _(gpsimd):
            for ti in range(NT):
                gpsimd.wait_ge(dsem, 16 * (ti + 1))
                if ti > 0:
                    gpsimd.wait_ge(vsem, ti)  # AB/AC/BC free after prev merge
                A = XT[ti][:, 0, :]
                Bv = XT[ti][:, 1, :]
                C = XT[ti][:, 2, :]
                nc.gpsimd.tensor_mul(AB[:, :], A, Bv)
                nc.gpsimd.tensor_mul(AC[:, :], A, C)
                nc.gpsimd.tensor_mul(BC[:, :], Bv, C).then_inc(gsem, 1)

        @block.sync
        def _(sync):
            for ti in range(NT):
                sl = slice(ti * FS, (ti + 1) * FS)
                sync.dma_start(XT[ti][:, :, :], X[:, :, sl]).then_inc(dsem, 16)
            for ti in range(NT):
                sl = slice(ti * FS, (ti + 1) * FS)
                sync.wait_ge(vsem, ti + 1)
                sync.dma_start(out_d[:, sl], ot[ti][:, :]).then_inc(osem, 16)
            sync.wait_ge(osem, 16 * NT)

        @block.scalar
        def _(scalar):
            for ti in range(NT):
                scalar.wait_ge(dsem, 16 * (ti + 1))
                A = XT[ti][:, 0, :]
                Bv = XT[ti][:, 1, :]
                C = XT[ti][:, 2, :]
                if ti > 0:
                    scalar.wait_ge(vsem, ti)  # Ra/s/Rb free after prev merge
                nc.scalar.square(AAs[ti][:, :], A)
                nc.scalar.square(BBs[ti][:, :], Bv)
                nc.scalar.square(CCs[ti][:, :], C).then_inc(asem, 1)
                nc.scalar.mul(Ra[:, :], AAs[ti][:, :], c[15])
                nc.scalar.mul(s[:, :], AAs[ti][:, :], c[5])
                nc.scalar.mul(Rb[:, :], BBs[ti][:, :], c[25]).then_inc(asem, 1)

        @block.vector
        def _(vector):
            for ti in range(NT):
                A = XT[ti][:, 0, :]
                Bv = XT[ti][:, 1, :]
                C = XT[ti][:, 2, :]
                D = XT[ti][:, 3, :]
                AA, BB, CC = AAs[ti][:, :], BBs[ti][:, :], CCs[ti][:, :]

                def stt(out, in0, sc, in1, op0=MUL, op1=ADD):
                    nc.vector.scalar_tensor_tensor(out, in0, sc, in1, op0, op1)

                vector.wait_ge(dsem, 16 * (ti + 1))
                # Horner-in-d chain: t = P2 + d*c_ddd
                nc.vector.tensor_scalar(t[:, :], D, c[34], c[14], MUL, ADD)
                stt(t[:, :], A, c[24], t[:, :])
                stt(t[:, :], Bv, c[30], t[:, :])
                stt(t[:, :], C, c[33], t[:, :])
                stt(t[:, :], D, 1.0, t[:, :], MUL, MUL)       # t *= d
                nc.vector.tensor_scalar(t[:, :], t[:, :], 1.0, c[4], MUL, ADD)
                stt(t[:, :], A, c[8], t[:, :])
                stt(t[:, :], Bv, c[11], t[:, :])
                stt(t[:, :], C, c[13], t[:, :])
                vector.wait_ge(asem, 2 * ti + 1)              # squares ready
                vector.wait_ge(gsem, ti + 1)                  # products ready
                stt(t[:, :], AA, c[18], t[:, :])
                stt(t[:, :], AB[:, :], c[21], t[:, :])
                stt(t[:, :], AC[:, :], c[23], t[:, :])
                stt(t[:, :], BB, c[27], t[:, :])
                stt(t[:, :], BC[:, :], c[29], t[:, :])
                stt(t[:, :], CC, c[32], t[:, :])
                stt(t[:, :], D, 1.0, t[:, :], MUL, MUL)       # t *= d
                nc.vector.tensor_scalar(t[:, :], t[:, :], 1.0, c[0], MUL, ADD)
                stt(t[:, :], A, c[1], t[:, :])
                stt(t[:, :], Bv, c[2], t[:, :])
                stt(t[:, :], C, c[3], t[:, :])
                # Ra = cubic-in-a row of quads (head from ACT)
                vector.wait_ge(asem, 2 * ti + 2)
                stt(Ra[:, :], AB[:, :], c[16], Ra[:, :])
                stt(Ra[:, :], AC[:, :], c[17], Ra[:, :])
                stt(Ra[:, :], BB, c[19], Ra[:, :])
                stt(Ra[:, :], BC[:, :], c[20], Ra[:, :])
                stt(Ra[:, :], CC, c[22], Ra[:, :])
                # s = P0 quad terms (head from ACT)
                stt(s[:, :], AB[:, :], c[6], s[:, :])
                stt(s[:, :], AC[:, :], c[7], s[:, :])
                stt(s[:, :], BB, c[9], s[:, :])
                stt(s[:, :], BC[:, :], c[10], s[:, :])
                stt(s[:, :], CC, c[12], s[:, :])
                # Rb (head from ACT)
                stt(Rb[:, :], BC[:, :], c[26], Rb[:, :])
                stt(Rb[:, :], CC, c[28], Rb[:, :])
                # merge
                stt(u[:, :], A, 1.0, Ra[:, :], MUL, MUL)      # u = a*Ra
                stt(t[:, :], u[:, :], 1.0, t[:, :])
                stt(u[:, :], Bv, 1.0, Rb[:, :], MUL, MUL)     # u = b*Rb
                stt(t[:, :], u[:, :], 1.0, t[:, :])
                stt(u[:, :], CC, c[31], C, MUL, MUL)          # u = c_ccc*CC*c
                stt(t[:, :], u[:, :], 1.0, t[:, :])
                nc.vector.scalar_tensor_tensor(
                    ot[ti][:, :], s[:, :], 1.0, t[:, :], MUL, ADD
                ).then_inc(vsem, 1)
    return nc


def _coeffs(weights, qz_loga):
    qz = qz_loga.astype(np.float64)
    z = np.clip(1.0 / (1.0 + np.exp(-qz)) * (ZETA - GAMMA) + GAMMA, 0.0, 1.0)
    return (z * weights.astype(np.float64)[:, 0]).astype(np.float32)


def kernel(obs, act, weights, qz_loga):
    from concourse.bass_utils import run_bass_kernel_spmd

    c = _coeffs(weights, qz_loga)
    x = np.concatenate([obs.astype(np.float32), act.astype(np.float32)], axis=1)
    dt = ml_dtypes.bfloat16 if COMPUTE == "bf16" else np.float32
    # per core: [P, 4, FTOT] with element (p, v, f) = x[i*R + p*FTOT + f, v]
    xp = np.ascontiguousarray(
        x.reshape(NCORES, P, FTOT, 4).transpose(0, 1, 3, 2)
    ).astype(dt)

    in_maps = [{"X": xp[i]} for i in range(NCORES)]

    nc = _build_nc(c)
    trace = bool(_CACHED.get("trace"))
    res = run_bass_kernel_spmd(nc, in_maps, core_ids=list(range(NCORES)),
                               trace=trace)
    _CACHED["last_results"] = res
    out = np.concatenate(
        [r["out"].reshape(R) for r in res.results]
    ).astype(np.float32)[:, None]
    return out
